# revision 24
# baseline (speedup 1.0000x reference)
"""Causal self-attention (B=4, N=2048, D=1024, single head) on 8 TRN2 NeuronCores.

Sharding: core c handles batch b = c//2, query shard h = c%2 with the
stride-2 interleave q_global = 2*j + h  (j = 0..1023).  The interleave makes
the causal-mask *tile structure* identical on every core (SPMD-uniform), so
fully-masked score tiles are skipped structurally while the residual
diagonal masking is data-driven (query-position tensor per core).

Because the attention is single-head (D_head == D_model), the four weight
matrices fold into two host-side products, removing the K and V projections
entirely:
  scores ~ Xq @ (Wq^T Wk) @ X^T + (Wk^T bq).X^T   (+ per-query terms that
                                                   softmax ignores)
  out    = [P @ X] @ (Wo Wv)^T / rowsum + (bo + Wo bv)

Per-core pipeline (f32 PSUM accumulation everywhere):
  GT[c,j]  = W_qk^T @ Xq + bgt   (bf16; evicted to fp8e4 pair layout)
  ST[k,j]  = X^T-pairs @ GT      (fp8 DoubleRow: 2 contraction rows/PE cell)
  E        = exp(ST/sqrt(D)) * causal_mask   (no max-sub: |scores/32| <~ 2)
  rowsum[j]= ones.T @ E          (PE reduction over k partitions)
  Z[c,j]   = X^T @ E             (bf16; eviction fused with *1/rowsum)
  OT[e,j]  = W_vo^T @ Z          (bf16) ; out = OT + (bo + Wo bv)

Loops are ordered so each stationary (lhsT) operand feeds several
back-to-back matmuls; PSUM evictions run on the Vector engine with the
biases/normalization fused in.  No collectives: each core receives exactly
the host-side shard it needs (measured 8-core AllGather here is ~100us/MB,
far too slow to beat recomputing the shared projections).
"""

import os
import numpy as np
import ml_dtypes

BF16 = ml_dtypes.bfloat16
FP8 = ml_dtypes.float8_e4m3

N_CORES = 8
B, N, D = 4, 2048, 1024
NQ = 1024           # queries per core
P = 128             # partitions
ET = D // P         # 8  e-tiles
CT_ = D // P        # 8  contraction tiles of D
KT_ALL = N // P     # 16 key tiles
JCW = 512           # free-dim chunk
NJC = NQ // JCW     # 2

_cache = {}


def _build():
    from concourse import bacc, tile, mybir
    import concourse.bass as bass

    f32 = mybir.dt.float32
    bf16 = mybir.dt.bfloat16
    fp8 = mybir.dt.float8e4
    DR = mybir.MatmulPerfMode.DoubleRow
    Exp = mybir.ActivationFunctionType.Exp
    is_ge = mybir.AluOpType.is_ge
    add = mybir.AluOpType.add
    mult = mybir.AluOpType.mult
    PSUM = bass.MemorySpace.PSUM

    SCL = float(1.0 / np.sqrt(np.float32(D)))
    nc = bacc.Bacc("TRN2", target_bir_lowering=False, debug=False,
                   num_devices=N_CORES)

    xtp_d = nc.declare_dram_parameter("xtp", [ET // 2, P, 2, N], fp8,
                                      isOutput=False)
    xtq_d = nc.declare_dram_parameter("xtq", [D, NQ], bf16, isOutput=False)
    wqk_d = nc.declare_dram_parameter("wqk", [D, D], bf16, isOutput=False)
    wvot_d = nc.declare_dram_parameter("wvot", [D, D], bf16, isOutput=False)
    xtok_d = nc.declare_dram_parameter("xtok", [N, D], bf16, isOutput=False)
    bgt_d = nc.declare_dram_parameter("bgt", [P, ET], f32, isOutput=False)
    bot_d = nc.declare_dram_parameter("bot", [P, ET], f32, isOutput=False)
    bqp_d = nc.declare_dram_parameter("bqpos", [P, NQ], f32, isOutput=False)
    kpt_d = nc.declare_dram_parameter("kpost", [P, KT_ALL], f32, isOutput=False)
    out_d = nc.declare_dram_parameter("out", [D, NQ], f32, isOutput=True)
    warm_d = nc.dram_tensor("warmdump", [1, 4], f32)

    with tile.TileContext(nc) as tc:
        with (
            tc.tile_pool(name="consts", bufs=1) as p_c,
            tc.tile_pool(name="w", bufs=10) as p_w,
            tc.tile_pool(name="qt", bufs=ET) as p_qt,
            tc.tile_pool(name="kt", bufs=ET) as p_kt,
            tc.tile_pool(name="v", bufs=KT_ALL) as p_v,
            tc.tile_pool(name="ps", bufs=6, space=PSUM) as p_ps,
            tc.tile_pool(name="rsps", bufs=2, space=PSUM) as p_rs,
        ):
            # PE warmup: ~20 dummy matmuls on memset data keep the PE busy
            # through the DMA-fill window so HAM un-throttles (K=8/8) before
            # the first real matmul; a 16B DMA anchors the chain against DCE.
            warm_a = p_c.tile([P, JCW], bf16, tag="warm_a")
            nc.gpsimd.memset(warm_a[:], 1.0)
            warm_s = p_c.tile([1, 4], f32, tag="warm_s")
            for g in range(2):
                wps = p_ps.tile([P, JCW], f32, tag="ps", name="ps")
                for r in range(10):
                    nc.tensor.matmul(wps[:], warm_a[:, :P], warm_a[:],
                                     start=(r == 0), stop=(r == 9))
                if g == 1:
                    nc.vector.tensor_copy(warm_s[:], wps[0:1, 0:4])
                    nc.sync.dma_start(warm_d[:], warm_s[:])

            # pair layout for fp8 DoubleRow: [p, s, x] = value at row 2*i... i.e.
            # qt_pair[i][p, s, n] = Q[e = i*256 + s*128 + p, n]
            gt_pair = [p_qt.tile([P, 2, NQ], fp8, tag="qt", name="qt")
                       for _ in range(ET // 2)]
            xtp_tiles = [p_kt.tile([P, 2, N], fp8, tag="kt", name="kt")
                         for _ in range(ET // 2)]
            xtok_tiles = [p_v.tile([P, D], bf16, tag="v", name="v")
                          for _ in range(KT_ALL)]

            def load_w(dram):
                ts = []
                for ct in range(CT_):
                    t = p_w.tile([P, D], bf16, tag="w", name="w")
                    eng = nc.sync if ct % 2 == 0 else nc.scalar
                    eng.dma_start(t[:], dram[ct * P:(ct + 1) * P, :])
                    ts.append(t)
                return ts

            with tc.tile_pool(name="xtq", bufs=CT_) as p_xtq:
                # ---- GT = W_qk^T @ Xq  (the only remaining projection on
                # the score path; K projection folded into W_qk host-side) ----
                wq = []
                xtq_tiles = []
                for ct in range(CT_):
                    t = p_w.tile([P, D], bf16, tag="w", name="w")
                    eng = nc.sync if ct % 2 == 0 else nc.scalar
                    eng.dma_start(t[:], wqk_d[ct * P:(ct + 1) * P, :])
                    wq.append(t)
                    t2 = p_xtq.tile([P, NQ], bf16, tag="xtq", name="xtq")
                    nc.gpsimd.dma_start(t2[:], xtq_d[ct * P:(ct + 1) * P, :])
                    xtq_tiles.append(t2)
                bgt_t = p_c.tile([P, ET], f32, tag="bgt")
                nc.scalar.dma_start(bgt_t[:], bgt_d[:, :])
                for i in range(ET // 2):
                    nc.scalar.dma_start(xtp_tiles[i][:], xtp_d[i])

                for et in range(ET):
                    pss = [p_ps.tile([P, JCW], f32, tag="ps", name="ps")
                           for _ in range(NJC)]
                    for ct in range(CT_):
                        for jc in range(NJC):
                            nc.tensor.matmul(
                                pss[jc][:],
                                wq[ct][:, et * P:(et + 1) * P],
                                xtq_tiles[ct][:, jc * JCW:(jc + 1) * JCW],
                                start=(ct == 0), stop=(ct == CT_ - 1))
                    for jc in range(NJC):
                        nc.vector.tensor_scalar_add(
                            gt_pair[et // 2][:, et % 2,
                                             jc * JCW:(jc + 1) * JCW],
                            pss[jc][:], bgt_t[:, et:et + 1])

                # ---- X in token-partition layout (for Z = X^T @ P^T) ----
                for kt in range(KT_ALL):
                    eng2 = nc.gpsimd if kt % 2 == 0 else nc.sync
                    eng2.dma_start(xtok_tiles[kt][:],
                                   xtok_d[kt * P:(kt + 1) * P, :])

            # W_vo = Wo @ Wv tiles + remaining consts
            wo = load_w(wvot_d)
            ones_col = p_c.tile([P, 1], bf16, tag="ones_col")
            nc.gpsimd.memset(ones_col[:], 1.0)
            ones_col_f32 = p_c.tile([1, P], f32, tag="ones_col_f32")
            nc.gpsimd.memset(ones_col_f32[:], 1.0)
            bot_t = p_c.tile([P, ET], f32, tag="bot")
            nc.scalar.dma_start(bot_t[:], bot_d[:, :])
            bqpos_t = p_c.tile([P, NQ], f32, tag="bqpos")
            nc.scalar.dma_start(bqpos_t[:], bqp_d[:, :])
            kpost_t = p_c.tile([P, KT_ALL], f32, tag="kpost")
            nc.scalar.dma_start(kpost_t[:], kpt_d[:, :])

            with (
                tc.tile_pool(name="exp", bufs=KT_ALL + ET + 1) as p_exp,
                tc.tile_pool(name="raw", bufs=2) as p_raw,
                tc.tile_pool(name="ctx", bufs=2 * ET + 1) as p_ctx,
                tc.tile_pool(name="of", bufs=4) as p_of,
                tc.tile_pool(name="brec", bufs=2) as p_brec,
                tc.tile_pool(name="recip", bufs=2) as p_recip,
            ):
                # jc=0 covers global queries [0,1024): keys < 1024 (kt 0..7).
                # jc=1 covers [1024,2048): all 16 kt; kt 0..7 unmasked there.
                def jcs_of(kt):
                    return (0, 1) if kt < 8 else (1,)

                # ---- scores + exp + mask + rowsum ----
                rs_ps = {jc: p_rs.tile([1, JCW], f32, tag="rsps", name="rsps")
                         for jc in range(NJC)}
                exps = {}
                for kt in range(KT_ALL):
                    sts = {}
                    for jc in jcs_of(kt):
                        sts[jc] = p_ps.tile([P, JCW], f32, tag="ps", name="ps")
                    for i in range(ET // 2):
                        for jc in jcs_of(kt):
                            nc.tensor.matmul(
                                sts[jc][:],
                                xtp_tiles[i][:, :, kt * P:(kt + 1) * P],
                                gt_pair[i][:, :, jc * JCW:(jc + 1) * JCW],
                                start=(i == 0), stop=(i == ET // 2 - 1),
                                perf_mode=DR)
                    for jc in jcs_of(kt):
                        ex_t = p_exp.tile([P, JCW], bf16, tag="exp",
                                          name="exp")
                        exps[(jc, kt)] = ex_t
                        ex = ex_t[:]
                        boundary = (kt >= 8 * jc)
                        if boundary:
                            raw = p_raw.tile([P, JCW], bf16, tag="raw",
                                             name="raw")
                            nc.scalar.activation(raw[:], sts[jc][:], Exp,
                                                 scale=SCL)
                            nc.vector.scalar_tensor_tensor(
                                ex,
                                bqpos_t[:, jc * JCW:(jc + 1) * JCW],
                                kpost_t[:, kt:kt + 1], raw[:],
                                is_ge, mult)
                        else:
                            nc.scalar.activation(ex, sts[jc][:], Exp,
                                                 scale=SCL)
                        nkt = 8 if jc == 0 else 16
                        nc.tensor.matmul(
                            rs_ps[jc][:], ones_col[:], ex,
                            start=(kt == 0), stop=(kt == nkt - 1))

                # ---- reciprocal of rowsums (DVE, overlaps Z ct=0) ----
                recips = {}
                for jc in range(NJC):
                    recip_t = p_recip.tile([1, JCW], f32, tag="recip",
                                           name="recip")
                    nc.vector.reciprocal(recip_t[:], rs_ps[jc][:])
                    recips[jc] = recip_t

                # ---- Z = X^T @ P^T (normalize fused into eviction) ----
                zs = {}
                brec = {}
                for ct in range(CT_):
                    cps = {jc: p_ps.tile([P, JCW], f32, tag="ps", name="ps")
                           for jc in range(NJC)}
                    for kt in range(KT_ALL):
                        for jc in jcs_of(kt):
                            nkt = 8 if jc == 0 else 16
                            nc.tensor.matmul(
                                cps[jc][:],
                                xtok_tiles[kt][:, ct * P:(ct + 1) * P],
                                exps[(jc, kt)][:],
                                start=(kt == 0), stop=(kt == nkt - 1))
                    if ct == 0:
                        # broadcast 1/rowsum across partitions via K=1 matmul
                        for jc in range(NJC):
                            br_ps = p_ps.tile([P, JCW], f32, tag="ps",
                                              name="ps")
                            nc.tensor.matmul(br_ps[:], ones_col_f32[:],
                                             recips[jc][:],
                                             start=True, stop=True)
                            bt = p_brec.tile([P, JCW], f32, tag="brec",
                                             name="brec")
                            nc.vector.tensor_copy(bt[:], br_ps[:])
                            brec[jc] = bt
                    for jc in range(NJC):
                        z_t = p_ctx.tile([P, JCW], bf16, tag="ctx",
                                         name="ctx")
                        nc.vector.tensor_tensor(z_t[:], cps[jc][:],
                                                brec[jc][:], mult)
                        zs[(jc, ct)] = z_t

                # ---- output projection + normalize + bias ----
                for et in range(ET):
                    opss = {jc: p_ps.tile([P, JCW], f32, tag="ps", name="ps")
                            for jc in range(NJC)}
                    for ct in range(CT_):
                        for jc in range(NJC):
                            nc.tensor.matmul(
                                opss[jc][:],
                                wo[ct][:, et * P:(et + 1) * P],
                                zs[(jc, ct)][:],
                                start=(ct == 0), stop=(ct == CT_ - 1))
                    for jc in range(NJC):
                        jsl = slice(jc * JCW, (jc + 1) * JCW)
                        of2 = p_of.tile([P, JCW], f32, tag="of", name="of")
                        nc.vector.tensor_scalar_add(of2[:], opss[jc][:],
                                                    bot_t[:, et:et + 1])
                        nc.sync.dma_start(out_d[et * P:(et + 1) * P, jsl],
                                          of2[:])

    nc.compile()
    return nc


def _prep_in_maps(X, Wq, bq, Wk, bk, Wv, bv, Wo, bo):
    wqk = np.ascontiguousarray(Wq.astype(np.float64).T
                               @ Wk.astype(np.float64)).astype(BF16)
    wvot = np.ascontiguousarray((Wo.astype(np.float64)
                                 @ Wv.astype(np.float64)).T).astype(BF16)
    bgt = np.ascontiguousarray(
        (Wk.astype(np.float64).T @ bq.astype(np.float64))
        .reshape(ET, P).T).astype(np.float32)
    bo_eff = (bo.astype(np.float64)
              + Wo.astype(np.float64) @ bv.astype(np.float64))
    bot = np.ascontiguousarray(
        bo_eff.reshape(ET, P).T).astype(np.float32)
    kpost = np.ascontiguousarray(
        np.arange(N, dtype=np.float32).reshape(KT_ALL, P).T)

    in_maps = []
    for c in range(N_CORES):
        b, h = c // 2, c % 2
        Xb = X[b]
        xtok = np.ascontiguousarray(Xb).astype(BF16)
        xtq = np.ascontiguousarray(Xb[h::2].T).astype(BF16)
        xtp = np.ascontiguousarray(
            Xb.T.reshape(ET // 2, 2, P, N).transpose(0, 2, 1, 3)
        ).astype(FP8)
        qpos = (2.0 * np.arange(NQ, dtype=np.float32) + h)
        bqpos = np.ascontiguousarray(
            np.broadcast_to(qpos[None, :], (P, NQ))).astype(np.float32)
        in_maps.append({
            "xtp": xtp, "xtq": xtq, "xtok": xtok,
            "wqk": wqk, "wvot": wvot,
            "bgt": bgt, "bot": bot,
            "bqpos": bqpos, "kpost": kpost,
        })
    return in_maps


last_exec_time_ns = None


def _ensure_ntff_hook():
    """Register the axon NTFF profile hook if the image's antenv lacks it."""
    try:
        from antenv.axon_hooks import get_axon_ntff_profile_hook  # noqa: F401
        return
    except ImportError:
        pass
    import sys
    import types
    mod = types.ModuleType("antenv.axon_hooks")
    mod._hook = None
    mod.set_axon_ntff_profile_hook = lambda h: setattr(mod, "_hook", h)
    mod.get_axon_ntff_profile_hook = lambda: mod._hook
    sys.modules["antenv.axon_hooks"] = mod
    try:
        import antenv
        antenv.axon_hooks = mod
    except ImportError:
        pass
    try:
        from trn_agent_boot.trn_boot import _ntff_profile_via_ctypes
        mod._hook = _ntff_profile_via_ctypes("/opt/axon/libaxon_pjrt.so")
    except Exception:
        pass


def kernel(X, Wq, bq, Wk, bk, Wv, bv, Wo, bo):
    global last_exec_time_ns
    from concourse.bass_utils import run_bass_kernel_spmd
    _ensure_ntff_hook()

    X = np.asarray(X, dtype=np.float32)
    args = [np.asarray(a, dtype=np.float32)
            for a in (Wq, bq, Wk, bk, Wv, bv, Wo, bo)]

    if "nc" not in _cache:
        _cache["nc"] = _build()
    nc = _cache["nc"]

    in_maps = _prep_in_maps(X, *args)
    kwargs = {}
    tmpdir = os.environ.get("KERNEL_TRACE_DIR")
    if tmpdir:
        kwargs = dict(trace=True, tmpdir=tmpdir)
    try:
        res = run_bass_kernel_spmd(nc, in_maps,
                                   core_ids=list(range(N_CORES)), **kwargs)
    except Exception:
        if not kwargs and not os.environ.get("BASS_TRACE"):
            raise
        # trace post-processing can fail (no artifact share, old .so);
        # the numeric result must not depend on it
        os.environ["BASS_NEVER_TRACE"] = "1"
        try:
            res = run_bass_kernel_spmd(nc, in_maps,
                                       core_ids=list(range(N_CORES)))
        finally:
            del os.environ["BASS_NEVER_TRACE"]
    last_exec_time_ns = res.exec_time_ns

    out = np.empty((B, N, D), dtype=np.float32)
    for c in range(N_CORES):
        b, h = c // 2, c % 2
        out[b, h::2, :] = np.asarray(res.results[c]["out"],
                                     dtype=np.float32).T
    return out


# revision 27
# speedup vs baseline: 1.0291x; 1.0291x over previous
"""Causal self-attention (B=4, N=2048, D=1024, single head) on 8 TRN2 NeuronCores.

Sharding: core c handles batch b = c//2, query shard h = c%2 with the
stride-2 interleave q_global = 2*j + h  (j = 0..1023).  The interleave makes
the causal-mask *tile structure* identical on every core (SPMD-uniform), so
fully-masked score tiles are skipped structurally while the residual
diagonal masking is data-driven (query-position tensor per core).

Because the attention is single-head (D_head == D_model), the four weight
matrices fold into two host-side products, removing the K and V projections
entirely:
  scores ~ Xq @ (Wq^T Wk) @ X^T + (Wk^T bq).X^T   (+ per-query terms that
                                                   softmax ignores)
  out    = [P @ X] @ (Wo Wv)^T / rowsum + (bo + Wo bv)

Per-core pipeline (f32 PSUM accumulation everywhere):
  GT[c,j]  = W_qk^T @ Xq + bgt   (bf16; evicted to fp8e4 pair layout)
  ST[k,j]  = X^T-pairs @ GT      (fp8 DoubleRow: 2 contraction rows/PE cell)
  E        = exp(ST/sqrt(D)) * causal_mask   (no max-sub: |scores/32| <~ 2)
  rowsum[j]= ones.T @ E          (PE reduction over k partitions)
  Z[c,j]   = X^T @ E             (bf16; eviction fused with *1/rowsum)
  OT[e,j]  = W_vo^T @ Z          (bf16) ; out = OT + (bo + Wo bv)

Loops are ordered so each stationary (lhsT) operand feeds several
back-to-back matmuls; PSUM evictions run on the Vector engine with the
biases/normalization fused in.  No collectives: each core receives exactly
the host-side shard it needs (measured 8-core AllGather here is ~100us/MB,
far too slow to beat recomputing the shared projections).
"""

import os
import numpy as np
import ml_dtypes

BF16 = ml_dtypes.bfloat16
FP8 = ml_dtypes.float8_e4m3

N_CORES = 8
B, N, D = 4, 2048, 1024
NQ = 1024           # queries per core
P = 128             # partitions
ET = D // P         # 8  e-tiles
CT_ = D // P        # 8  contraction tiles of D
KT_ALL = N // P     # 16 key tiles
JCW = 512           # free-dim chunk
NJC = NQ // JCW     # 2

_cache = {}


def _build():
    from concourse import bacc, tile, mybir
    import concourse.bass as bass

    f32 = mybir.dt.float32
    bf16 = mybir.dt.bfloat16
    fp8 = mybir.dt.float8e4
    DR = mybir.MatmulPerfMode.DoubleRow
    Exp = mybir.ActivationFunctionType.Exp
    is_ge = mybir.AluOpType.is_ge
    add = mybir.AluOpType.add
    mult = mybir.AluOpType.mult
    PSUM = bass.MemorySpace.PSUM

    SCL = float(1.0 / np.sqrt(np.float32(D)))
    nc = bacc.Bacc("TRN2", target_bir_lowering=False, debug=False,
                   num_devices=N_CORES)

    xtp_d = nc.declare_dram_parameter("xtp", [ET // 2, P, 2, N], fp8,
                                      isOutput=False)
    xtq_d = nc.declare_dram_parameter("xtq", [D, NQ], bf16, isOutput=False)
    wqk_d = nc.declare_dram_parameter("wqk", [D, D], bf16, isOutput=False)
    wvot_d = nc.declare_dram_parameter("wvot", [D, D], bf16, isOutput=False)
    xtok_d = nc.declare_dram_parameter("xtok", [N, D], bf16, isOutput=False)
    bgt_d = nc.declare_dram_parameter("bgt", [P, ET], f32, isOutput=False)
    bot_d = nc.declare_dram_parameter("bot", [P, ET], f32, isOutput=False)
    bqp_d = nc.declare_dram_parameter("bqpos", [P, NQ], f32, isOutput=False)
    kpt_d = nc.declare_dram_parameter("kpost", [P, KT_ALL], f32, isOutput=False)
    out_d = nc.declare_dram_parameter("out", [D, NQ], f32, isOutput=True)

    with tile.TileContext(nc) as tc:
        with (
            tc.tile_pool(name="consts", bufs=1) as p_c,
            tc.tile_pool(name="w", bufs=10) as p_w,
            tc.tile_pool(name="qt", bufs=ET) as p_qt,
            tc.tile_pool(name="kt", bufs=ET) as p_kt,
            tc.tile_pool(name="v", bufs=KT_ALL) as p_v,
            tc.tile_pool(name="ps", bufs=6, space=PSUM) as p_ps,
            tc.tile_pool(name="rsps", bufs=2, space=PSUM) as p_rs,
        ):
            # pair layout for fp8 DoubleRow: [p, s, x] = value at row 2*i... i.e.
            # qt_pair[i][p, s, n] = Q[e = i*256 + s*128 + p, n]
            gt_pair = [p_qt.tile([P, 2, NQ], fp8, tag="qt", name="qt")
                       for _ in range(ET // 2)]
            xtp_tiles = [p_kt.tile([P, 2, N], fp8, tag="kt", name="kt")
                         for _ in range(ET // 2)]
            xtok_tiles = [p_v.tile([P, D], bf16, tag="v", name="v")
                          for _ in range(KT_ALL)]

            def load_w(dram):
                ts = []
                for ct in range(CT_):
                    t = p_w.tile([P, D], bf16, tag="w", name="w")
                    eng = nc.sync if ct % 2 == 0 else nc.scalar
                    eng.dma_start(t[:], dram[ct * P:(ct + 1) * P, :])
                    ts.append(t)
                return ts

            with tc.tile_pool(name="xtq", bufs=CT_) as p_xtq:
                # ---- GT = W_qk^T @ Xq  (the only remaining projection on
                # the score path; K projection folded into W_qk host-side) ----
                wq = []
                xtq_tiles = []
                for ct in range(CT_):
                    t = p_w.tile([P, D], bf16, tag="w", name="w")
                    eng = nc.sync if ct % 2 == 0 else nc.scalar
                    eng.dma_start(t[:], wqk_d[ct * P:(ct + 1) * P, :])
                    wq.append(t)
                    t2 = p_xtq.tile([P, NQ], bf16, tag="xtq", name="xtq")
                    nc.gpsimd.dma_start(t2[:], xtq_d[ct * P:(ct + 1) * P, :])
                    xtq_tiles.append(t2)
                bgt_t = p_c.tile([P, ET], f32, tag="bgt")
                nc.scalar.dma_start(bgt_t[:], bgt_d[:, :])
                for i in range(ET // 2):
                    nc.scalar.dma_start(xtp_tiles[i][:], xtp_d[i])

                for et in range(ET):
                    pss = [p_ps.tile([P, JCW], f32, tag="ps", name="ps")
                           for _ in range(NJC)]
                    for ct in range(CT_):
                        for jc in range(NJC):
                            nc.tensor.matmul(
                                pss[jc][:],
                                wq[ct][:, et * P:(et + 1) * P],
                                xtq_tiles[ct][:, jc * JCW:(jc + 1) * JCW],
                                start=(ct == 0), stop=(ct == CT_ - 1))
                    for jc in range(NJC):
                        nc.vector.tensor_scalar_add(
                            gt_pair[et // 2][:, et % 2,
                                             jc * JCW:(jc + 1) * JCW],
                            pss[jc][:], bgt_t[:, et:et + 1])

                # ---- X in token-partition layout (for Z = X^T @ P^T) ----
                for kt in range(KT_ALL):
                    eng2 = nc.gpsimd if kt % 2 == 0 else nc.sync
                    eng2.dma_start(xtok_tiles[kt][:],
                                   xtok_d[kt * P:(kt + 1) * P, :])

            # W_vo = Wo @ Wv tiles + remaining consts
            wo = load_w(wvot_d)
            ones_col = p_c.tile([P, 1], bf16, tag="ones_col")
            nc.gpsimd.memset(ones_col[:], 1.0)
            ones_col_f32 = p_c.tile([1, P], f32, tag="ones_col_f32")
            nc.gpsimd.memset(ones_col_f32[:], 1.0)
            bot_t = p_c.tile([P, ET], f32, tag="bot")
            nc.scalar.dma_start(bot_t[:], bot_d[:, :])
            bqpos_t = p_c.tile([P, NQ], f32, tag="bqpos")
            nc.scalar.dma_start(bqpos_t[:], bqp_d[:, :])
            kpost_t = p_c.tile([P, KT_ALL], f32, tag="kpost")
            nc.scalar.dma_start(kpost_t[:], kpt_d[:, :])

            with (
                tc.tile_pool(name="exp", bufs=KT_ALL + ET + 1) as p_exp,
                tc.tile_pool(name="raw", bufs=2) as p_raw,
                tc.tile_pool(name="ctx", bufs=2 * ET + 1) as p_ctx,
                tc.tile_pool(name="of", bufs=4) as p_of,
                tc.tile_pool(name="brec", bufs=2) as p_brec,
                tc.tile_pool(name="recip", bufs=2) as p_recip,
            ):
                # jc=0 covers global queries [0,1024): keys < 1024 (kt 0..7).
                # jc=1 covers [1024,2048): all 16 kt; kt 0..7 unmasked there.
                def jcs_of(kt):
                    return (0, 1) if kt < 8 else (1,)

                # ---- scores + exp + mask + rowsum ----
                rs_ps = {jc: p_rs.tile([1, JCW], f32, tag="rsps", name="rsps")
                         for jc in range(NJC)}
                exps = {}
                for kt in range(KT_ALL):
                    sts = {}
                    for jc in jcs_of(kt):
                        sts[jc] = p_ps.tile([P, JCW], f32, tag="ps", name="ps")
                    for i in range(ET // 2):
                        for jc in jcs_of(kt):
                            nc.tensor.matmul(
                                sts[jc][:],
                                xtp_tiles[i][:, :, kt * P:(kt + 1) * P],
                                gt_pair[i][:, :, jc * JCW:(jc + 1) * JCW],
                                start=(i == 0), stop=(i == ET // 2 - 1),
                                perf_mode=DR)
                    for jc in jcs_of(kt):
                        ex_t = p_exp.tile([P, JCW], bf16, tag="exp",
                                          name="exp")
                        exps[(jc, kt)] = ex_t
                        ex = ex_t[:]
                        boundary = (kt >= 8 * jc)
                        if boundary:
                            raw = p_raw.tile([P, JCW], bf16, tag="raw",
                                             name="raw")
                            nc.scalar.activation(raw[:], sts[jc][:], Exp,
                                                 scale=SCL)
                            nc.vector.scalar_tensor_tensor(
                                ex,
                                bqpos_t[:, jc * JCW:(jc + 1) * JCW],
                                kpost_t[:, kt:kt + 1], raw[:],
                                is_ge, mult)
                        else:
                            nc.scalar.activation(ex, sts[jc][:], Exp,
                                                 scale=SCL)
                        nkt = 8 if jc == 0 else 16
                        nc.tensor.matmul(
                            rs_ps[jc][:], ones_col[:], ex,
                            start=(kt == 0), stop=(kt == nkt - 1))

                # ---- reciprocal of rowsums (DVE, overlaps Z ct=0) ----
                recips = {}
                for jc in range(NJC):
                    recip_t = p_recip.tile([1, JCW], f32, tag="recip",
                                           name="recip")
                    nc.vector.reciprocal(recip_t[:], rs_ps[jc][:])
                    recips[jc] = recip_t

                # ---- Z = X^T @ P^T (normalize fused into eviction) ----
                zs = {}
                brec = {}
                for ct in range(CT_):
                    cps = {jc: p_ps.tile([P, JCW], f32, tag="ps", name="ps")
                           for jc in range(NJC)}
                    for kt in range(KT_ALL):
                        for jc in jcs_of(kt):
                            nkt = 8 if jc == 0 else 16
                            nc.tensor.matmul(
                                cps[jc][:],
                                xtok_tiles[kt][:, ct * P:(ct + 1) * P],
                                exps[(jc, kt)][:],
                                start=(kt == 0), stop=(kt == nkt - 1))
                    if ct == 0:
                        # broadcast 1/rowsum across partitions via K=1 matmul
                        for jc in range(NJC):
                            br_ps = p_ps.tile([P, JCW], f32, tag="ps",
                                              name="ps")
                            nc.tensor.matmul(br_ps[:], ones_col_f32[:],
                                             recips[jc][:],
                                             start=True, stop=True)
                            bt = p_brec.tile([P, JCW], f32, tag="brec",
                                             name="brec")
                            nc.vector.tensor_copy(bt[:], br_ps[:])
                            brec[jc] = bt
                    for jc in range(NJC):
                        z_t = p_ctx.tile([P, JCW], bf16, tag="ctx",
                                         name="ctx")
                        nc.vector.tensor_tensor(z_t[:], cps[jc][:],
                                                brec[jc][:], mult)
                        zs[(jc, ct)] = z_t

                # ---- output projection + normalize + bias ----
                for et in range(ET):
                    opss = {jc: p_ps.tile([P, JCW], f32, tag="ps", name="ps")
                            for jc in range(NJC)}
                    for ct in range(CT_):
                        for jc in range(NJC):
                            nc.tensor.matmul(
                                opss[jc][:],
                                wo[ct][:, et * P:(et + 1) * P],
                                zs[(jc, ct)][:],
                                start=(ct == 0), stop=(ct == CT_ - 1))
                    for jc in range(NJC):
                        jsl = slice(jc * JCW, (jc + 1) * JCW)
                        of2 = p_of.tile([P, JCW], f32, tag="of", name="of")
                        nc.vector.tensor_scalar_add(of2[:], opss[jc][:],
                                                    bot_t[:, et:et + 1])
                        nc.sync.dma_start(out_d[et * P:(et + 1) * P, jsl],
                                          of2[:])

    nc.compile()
    return nc


def _prep_in_maps(X, Wq, bq, Wk, bk, Wv, bv, Wo, bo):
    wqk = np.ascontiguousarray(Wq.astype(np.float64).T
                               @ Wk.astype(np.float64)).astype(BF16)
    wvot = np.ascontiguousarray((Wo.astype(np.float64)
                                 @ Wv.astype(np.float64)).T).astype(BF16)
    bgt = np.ascontiguousarray(
        (Wk.astype(np.float64).T @ bq.astype(np.float64))
        .reshape(ET, P).T).astype(np.float32)
    bo_eff = (bo.astype(np.float64)
              + Wo.astype(np.float64) @ bv.astype(np.float64))
    bot = np.ascontiguousarray(
        bo_eff.reshape(ET, P).T).astype(np.float32)
    kpost = np.ascontiguousarray(
        np.arange(N, dtype=np.float32).reshape(KT_ALL, P).T)

    in_maps = []
    for c in range(N_CORES):
        b, h = c // 2, c % 2
        Xb = X[b]
        xtok = np.ascontiguousarray(Xb).astype(BF16)
        xtq = np.ascontiguousarray(Xb[h::2].T).astype(BF16)
        xtp = np.ascontiguousarray(
            Xb.T.reshape(ET // 2, 2, P, N).transpose(0, 2, 1, 3)
        ).astype(FP8)
        qpos = (2.0 * np.arange(NQ, dtype=np.float32) + h)
        bqpos = np.ascontiguousarray(
            np.broadcast_to(qpos[None, :], (P, NQ))).astype(np.float32)
        in_maps.append({
            "xtp": xtp, "xtq": xtq, "xtok": xtok,
            "wqk": wqk, "wvot": wvot,
            "bgt": bgt, "bot": bot,
            "bqpos": bqpos, "kpost": kpost,
        })
    return in_maps


last_exec_time_ns = None


def _ensure_ntff_hook():
    """Register the axon NTFF profile hook if the image's antenv lacks it."""
    try:
        from antenv.axon_hooks import get_axon_ntff_profile_hook  # noqa: F401
        return
    except ImportError:
        pass
    import sys
    import types
    mod = types.ModuleType("antenv.axon_hooks")
    mod._hook = None
    mod.set_axon_ntff_profile_hook = lambda h: setattr(mod, "_hook", h)
    mod.get_axon_ntff_profile_hook = lambda: mod._hook
    sys.modules["antenv.axon_hooks"] = mod
    try:
        import antenv
        antenv.axon_hooks = mod
    except ImportError:
        pass
    try:
        from trn_agent_boot.trn_boot import _ntff_profile_via_ctypes
        mod._hook = _ntff_profile_via_ctypes("/opt/axon/libaxon_pjrt.so")
    except Exception:
        pass


def kernel(X, Wq, bq, Wk, bk, Wv, bv, Wo, bo):
    global last_exec_time_ns
    from concourse.bass_utils import run_bass_kernel_spmd
    _ensure_ntff_hook()

    X = np.asarray(X, dtype=np.float32)
    args = [np.asarray(a, dtype=np.float32)
            for a in (Wq, bq, Wk, bk, Wv, bv, Wo, bo)]

    if "nc" not in _cache:
        _cache["nc"] = _build()
    nc = _cache["nc"]

    in_maps = _prep_in_maps(X, *args)
    kwargs = {}
    tmpdir = os.environ.get("KERNEL_TRACE_DIR")
    if tmpdir:
        kwargs = dict(trace=True, tmpdir=tmpdir)
    try:
        res = run_bass_kernel_spmd(nc, in_maps,
                                   core_ids=list(range(N_CORES)), **kwargs)
    except Exception:
        if not kwargs and not os.environ.get("BASS_TRACE"):
            raise
        # trace post-processing can fail (no artifact share, old .so);
        # the numeric result must not depend on it
        os.environ["BASS_NEVER_TRACE"] = "1"
        try:
            res = run_bass_kernel_spmd(nc, in_maps,
                                       core_ids=list(range(N_CORES)))
        finally:
            del os.environ["BASS_NEVER_TRACE"]
    last_exec_time_ns = res.exec_time_ns

    out = np.empty((B, N, D), dtype=np.float32)
    for c in range(N_CORES):
        b, h = c // 2, c % 2
        out[b, h::2, :] = np.asarray(res.results[c]["out"],
                                     dtype=np.float32).T
    return out
